# revision 1
# baseline (speedup 1.0000x reference)
import sys

sys.path.insert(0, "/opt/trn_rl_repo")

import numpy as np

import concourse.bacc as bacc
import concourse.bass as bass
import concourse.mybir as mybir
import concourse.tile as tile
from concourse.bass_utils import run_bass_kernel_spmd

F32 = mybir.dt.float32
F16 = mybir.dt.float16

N, M, G, A, H = 20000, 48, 16, 64, 16
NCORES = 8
NL = N // NCORES  # 2500 atoms per core
NLP = 2560  # padded per-core atoms (5 groups of 512)
CW = 128  # per-atom cols: [a 0:64 | gs 64:80 | gv d0 80:96 d1 96:112 d2 112:128]
NB2 = 512  # group size (atoms) = 256 pairs
NFL = 8  # pairs per psum flush
SECA = 0  # d0 (bands 0/64) or d1 (bands 32/96) pair sections
SECB = 48 * 32  # d2 (bands 32/96) pair sections
SECI = 2 * 48 * 32  # identity section (bands 0/64)
WCOLS = 2 * 48 * 32 + 48

_nc_cache = {}


def _build(nlp=NLP, sim=False, reps=1):
    """Per-core Bass program (all matmuls fp16), software-pipelined.

    Input acat [M, nlp, CW] half-major on atoms (atom i<nlp/2 at slot i,
    else slot i; pairs are (i, nlp/2+i), halves on partition bases 0/64).

    Per super-iteration: stage-1 chunk k of group g (32 pairs: input DMA,
    8 pair-MMs x4 flushes, evac to vbig) interleaved with stage-2 k-pass
    of group g-1 (8 channels {16c+2k+odd}: 4 MMs each into psum2 banks
    [d0|S|d1|d2], ACT square, ACT S-copy, Pool adds into ovs), then
    group-tail: DVE block-transpose ovs->tts, 16 SWDGE cast-DMAs out.
    """
    assert nlp % NB2 == 0
    nc = bacc.Bacc("TRN2", target_bir_lowering=False)
    ac_d = nc.declare_dram_parameter("acat", [M, nlp, CW], F16, isOutput=False)
    w_d = nc.declare_dram_parameter("aghw", [128, WCOLS], F16, isOutput=False)
    out_d = nc.declare_dram_parameter("out", [nlp, A * G + A * H], F32, isOutput=True)

    Sq = mybir.ActivationFunctionType.Square
    ngroups = nlp // NB2
    gp = NB2 // 2  # pairs per group (256)
    half = nlp // 2

    with tile.TileContext(nc) as tc:
        with (
            tc.tile_pool(name="singles", bufs=1) as singles,
            tc.tile_pool(name="ain", bufs=6) as ain_pool,
            tc.tile_pool(name="vbig", bufs=2) as vbig_pool,
            tc.tile_pool(name="sq", bufs=2) as sq_pool,
            tc.tile_pool(name="ovs", bufs=2) as ovs_pool,
            tc.tile_pool(name="tt", bufs=2) as tt_pool,
            tc.tile_pool(name="psum1", bufs=2, space="PSUM") as p1_pool,
            tc.tile_pool(name="psum2", bufs=3, space="PSUM") as p2_pool,
        ):
            aghw = singles.tile([128, WCOLS], F16)
            nc.sync.dma_start(out=aghw[:, :], in_=w_d[:, :])

            def stage1_chunk(g2, ch, vbig):
                cp0 = g2 * gp + ch * 32
                acs = ain_pool.tile([128, 32 * CW], F16)
                for b in range(2):
                    eng = nc.sync if b == 0 else nc.scalar
                    eng.dma_start(
                        out=acs[64 * b : 64 * b + M, :].rearrange(
                            "p (j c) -> p j c", j=32
                        ),
                        in_=ac_d[:, b * half + cp0 : b * half + cp0 + 32, :],
                    )
                for fl in range(4):
                    psum1 = p1_pool.tile([128, 512], F32)
                    for jj in range(NFL):
                        c0 = (fl * NFL + jj) * CW
                        for par in range(2):
                            pb = 64 * par
                            nc.tensor.matmul(
                                out=psum1[pb : pb + 64, jj * A : (jj + 1) * A],
                                lhsT=acs[pb : pb + M, c0 + 64 : c0 + 128],
                                rhs=acs[pb : pb + M, c0 : c0 + 64],
                                start=True,
                                stop=True,
                            )
                    vcol = (ch * 32 + fl * NFL) * A
                    nc.vector.tensor_copy(
                        out=vbig[:, vcol : vcol + 512], in_=psum1[:, 0:512]
                    )

            def stage2_pass(k, vbig, ovs):
                vb3 = vbig[:, :].rearrange("p (n a) -> p n a", a=A)
                nsl = slice(0, 256)
                for P in range(2):  # atom half
                    rb = 64 * P
                    psum2 = p2_pool.tile([128, 1024], F32)
                    for c in range(4):
                        for odd in (1, 0):  # odd first: zeros overwritten
                            a_ch = 16 * c + 2 * k + odd
                            wc = SECA + (a_ch // 2) * 48
                            wc2 = SECB + (a_ch // 2) * 48
                            if odd:
                                wsl = slice(wc + 16, wc + 48)
                                wsl2 = slice(wc2 + 16, wc2 + 48)
                                isl = slice(SECI + 16, SECI + 48)
                                osz = 32
                            else:
                                wsl = slice(wc, wc + 16)
                                wsl2 = slice(wc2, wc2 + 16)
                                isl = slice(SECI, SECI + 16)
                                osz = 16
                            acol = slice(a_ch, a_ch + 1)
                            orows = slice(32 * c, 32 * c + osz)
                            nc.tensor.matmul(
                                out=psum2[orows, 0:256],
                                lhsT=aghw[rb : rb + 32, wsl],
                                rhs=vb3[rb : rb + 32, nsl, acol],
                                start=True,
                                stop=True,
                                tile_position=(rb, 32 * c),
                            )
                            nc.tensor.matmul(
                                out=psum2[orows, 256:512],
                                lhsT=aghw[rb : rb + 32, isl],
                                rhs=vb3[rb : rb + 32, nsl, acol],
                                start=True,
                                stop=True,
                                tile_position=(rb, 32 * c),
                            )
                            nc.tensor.matmul(
                                out=psum2[orows, 512:768],
                                lhsT=aghw[rb + 32 : rb + 64, wsl],
                                rhs=vb3[rb + 32 : rb + 64, nsl, acol],
                                start=True,
                                stop=True,
                                tile_position=(rb + 32, 32 * c),
                            )
                            nc.tensor.matmul(
                                out=psum2[orows, 768:1024],
                                lhsT=aghw[rb + 32 : rb + 64, wsl2],
                                rhs=vb3[rb + 32 : rb + 64, nsl, acol],
                                start=True,
                                stop=True,
                                tile_position=(rb + 32, 32 * c),
                            )
                    sq = sq_pool.tile([128, 1024], F16)
                    nc.scalar.activation(out=sq[:, :], in_=psum2[:, :], func=Sq)
                    ov = ovs[P]
                    nc.scalar.copy(
                        out=ov[:, 0:2048].rearrange(
                            "p (nb kk q) -> p nb kk q", nb=8, kk=8
                        )[:, :, k, :],
                        in_=psum2[:, 256:512].rearrange("p (nb q) -> p nb q", nb=8),
                    )
                    vdst = ov[:, 2048:4096].rearrange(
                        "p (nb kk q) -> p nb kk q", nb=8, kk=8
                    )[:, :, k, :]
                    sq4 = sq[:, :].rearrange("p (d nb q) -> p d nb q", d=4, nb=8)
                    nc.gpsimd.tensor_add(vdst, sq4[:, 0], sq4[:, 2])
                    nc.gpsimd.tensor_add(vdst, vdst, sq4[:, 3])

            def group_tail(g2, ovs, tts):
                for P in range(2):
                    tpt = tts[P]
                    nc.vector.transpose(out=tpt[:, :], in_=ovs[P][:, :])
                    rb0 = P * half + 256 * g2
                    for br in range(2):
                        for c in range(4):
                            cb0 = br * 1024 + 256 * c
                            nc.gpsimd.dma_start(
                                out=out_d[rb0 : rb0 + 256, cb0 : cb0 + 256].rearrange(
                                    "(nb nl) kq -> nl nb kq", nb=8
                                ),
                                in_=tpt[
                                    32 * c : 32 * c + 32, br * 2048 : (br + 1) * 2048
                                ].rearrange("p (nb kq) -> p nb kq", nb=8),
                            )

            for rep in range(reps):
                prev = None
                for g2 in range(ngroups + 1):
                    cur = None
                    if g2 < ngroups:
                        vbig = vbig_pool.tile([128, gp * A], F16)
                        ovs_e = ovs_pool.tile([128, 4096], F16)
                        ovs_o = ovs_pool.tile([128, 4096], F16)
                        tt_e = tt_pool.tile([128, 4096], F16)
                        tt_o = tt_pool.tile([128, 4096], F16)
                        if sim:
                            nc.vector.memset(ovs_e[:, :], 0.0)
                            nc.vector.memset(ovs_o[:, :], 0.0)
                        cur = (g2, vbig, [ovs_e, ovs_o], [tt_e, tt_o])
                    for k in range(8):
                        if cur is not None:
                            stage1_chunk(g2, k, cur[1])
                        if prev is not None:
                            stage2_pass(k, prev[1], prev[2])
                    if prev is not None:
                        group_tail(prev[0], prev[2], prev[3])
                    prev = cur
    nc.compile()
    return nc


def _get_nc():
    if "nc" not in _nc_cache:
        _nc_cache["nc"] = _build()
    return _nc_cache["nc"]


def _prep_core(a, gs, gv, nlp=NLP):
    """[nl, M, *] fp32 slices -> half-major m-major fp16 [M, nlp, CW].

    Atom i < nlp/2 stays at slot i; atom nlp/2 + j at slot nlp/2 + j
    (i.e. plain order, zero-padded to nlp)."""
    nl = a.shape[0]
    acat = np.zeros((M, nlp, CW), np.float16)
    acat[:, 0:nl, 0:A] = np.transpose(a, (1, 0, 2))
    acat[:, 0:nl, A : A + G] = np.transpose(gs, (1, 0, 2))
    gvt = np.transpose(gv, (1, 0, 2, 3))  # [M, nl, G, 3]
    for d in range(3):
        acat[:, 0:nl, 80 + 16 * d : 96 + 16 * d] = gvt[:, :, :, d]
    return acat


def _prep_w(agh):
    """agh [A,G,H] fp32 -> aghw [128, WCOLS] fp16.

    32-row bands at partition bases {0,64} (= [S|d0] operand rows) and
    {32,96} (= [d1|d2] rows). Per channel-pair cols [E(16)|Z(16)|O(16)]
    (odd lhsT = cols 16:48, leading 16 zero cols):
      band 0/64,  SECA: rows 16-31 = agh (d0 weight, top rows zero)
      band 32/96, SECA: rows 0-15 = agh (d1), SECB: rows 16-31 = agh (d2)
      band 0/64,  SECI: rows 0-15 = identity (S placement)
    """
    aghw = np.zeros((128, WCOLS), np.float16)
    ag = np.asarray(agh, np.float32)  # [A, G, H]
    for p in range(32):
        for a_ch, co in ((2 * p, 0), (2 * p + 1, 32)):
            w = ag[a_ch]
            for b0 in (0, 64):
                aghw[b0 + 16 : b0 + 32, SECA + 48 * p + co : SECA + 48 * p + co + 16] = w
            for b0 in (32, 96):
                aghw[b0 : b0 + 16, SECA + 48 * p + co : SECA + 48 * p + co + 16] = w
                aghw[b0 + 16 : b0 + 32, SECB + 48 * p + co : SECB + 48 * p + co + 16] = w
    eye = np.eye(16, dtype=np.float16)
    for b0 in (0, 64):
        aghw[b0 : b0 + 16, SECI : SECI + 16] = eye
        aghw[b0 : b0 + 16, SECI + 32 : SECI + 48] = eye
    return aghw


def kernel(a, gs, gv, agh):
    a = np.asarray(a, np.float32)
    gs = np.asarray(gs, np.float32)
    gv = np.asarray(gv, np.float32)
    aghw = _prep_w(agh)
    nc = _get_nc()
    in_maps = []
    for c in range(NCORES):
        sl = slice(c * NL, (c + 1) * NL)
        acat = _prep_core(a[sl], gs[sl], gv[sl])
        in_maps.append({"acat": acat, "aghw": aghw})
    res = run_bass_kernel_spmd(nc, in_maps, list(range(NCORES))).results
    return np.concatenate([res[c]["out"][:NL] for c in range(NCORES)], axis=0)

